# revision 31
# baseline (speedup 1.0000x reference)
"""Bhattacharyya coefficient kernel for Trainium2 (8 NeuronCores, SPMD).

out[n,0,i,j] = (1/k^2) * sum_{c,p,q} w[c] * sqrt(x[n,c,i+p,j+q] * z[n,c,p,q])

Data-parallel over batch: 2 samples per core. Per sample:
  1. ACT: sx = sqrt(x) (bf16), szw = w/k^2 * sqrt(z) (bf16).
  2. TensorE: plane[t, y] = sum_c szw[c, t] * sx[c, y] for the 64 taps
     t = 8p+q and all 63*63 image pixels y (K=256 in two 128-chunks
     accumulated in PSUM, M=64 taps, N in blocks of <=512).
  3. Evict PSUM -> SBUF (bf16, DVE), dump plane pieces to DRAM scratch.
  4. Gather back with per-tap shifted offsets (flat DRAM AP):
     A[t, u] = plane[t, u + 63*(t>>3) + (t&7)], turning the tap-sum into a
     pure partition reduction; chunked so each gather only needs the dumps
     that cover its column range.
  5. TensorE ones-matmul: o[u] = sum_t A[t, u]; evict; out[i,j] = o[63i+j].

The x loads are split into pieces (1MB, 0.79MB, 0.2MB per c-chunk) issued
up-front on the Sync HWDGE ring; the final tiny piece keeps the end-of-
kernel serial chain (last DMA -> sqrt -> matmul -> evict -> dump -> gather
-> reduce -> out) short.
"""

import numpy as np

import concourse.bacc as bacc
import concourse.bass as bass
import concourse.mybir as mybir
from concourse import tile
from concourse.bass_utils import run_bass_kernel_spmd

N, C, KS, MS = 16, 256, 8, 63
MO = MS - KS + 1            # 56
F = MS * MS                 # 3969
L = (MO - 1) * MS + MO + 2  # 3522 (even; last needed flat index is 63*55+55)
NCORES = 8
SPC = N // NCORES           # samples per core
BLK = 512
AF = mybir.ActivationFunctionType
f32 = mybir.dt.float32
bf16 = mybir.dt.bfloat16

# x staging pieces (start block, n blocks): 4 + 3 + 1
PIECES = [(0, 4), (4, 3), (7, 1)]
# stage-2 gather chunks (u0, len): chunk i>=1 reads scratch region 1
CHUNKS = [(0, 2 * BLK), (2 * BLK, 2 * BLK), (4 * BLK, 2 * BLK),
          (6 * BLK, L - 6 * BLK)]

_CACHE = {}


def _build():
    nc = bacc.Bacc("TRN2", target_bir_lowering=False, debug=False)
    z_in = nc.declare_dram_parameter("z", [SPC, C, KS, KS], f32, isOutput=False)
    x_in = nc.declare_dram_parameter("x", [SPC, C, MS, MS], f32, isOutput=False)
    w_in = nc.declare_dram_parameter("w", [C], f32, isOutput=False)
    out = nc.declare_dram_parameter("out", [SPC, 1, MO, MO], f32, isOutput=True)

    # DRAM scratch: one tensor per gather chunk (per sample) so Tile's
    # tensor-granular DRAM dependency tracking makes each gather wait only
    # for the dumps covering its own column range. Chunk tensor ci holds
    # plane cols [1024*ci, 1024*ci + pitch).
    PITS = [3 * BLK, 3 * BLK, 3 * BLK, 904]
    scs = [
        [nc.dram_tensor(f"pl_sc{ci}_{s}", [64, PITS[ci]], bf16)
         for ci in range(4)]
        for s in range(SPC)
    ]

    xflat = x_in.rearrange("s (k c) h w -> s k c (h w)", c=128)  # [SPC,2,128,F]

    with tile.TileContext(nc) as tc:
        with (
            tc.tile_pool(name="xstage", bufs=11) as xstage,
            tc.tile_pool(name="sxq", bufs=7) as sxq,
            tc.tile_pool(name="zpool", bufs=2) as zpool,
            tc.tile_pool(name="plane", bufs=2) as plane,
            tc.tile_pool(name="gath", bufs=6) as gath,
            tc.tile_pool(name="opool", bufs=1) as opool,
            tc.tile_pool(name="psum", bufs=2, space="PSUM") as psum,
            tc.tile_pool(name="psum2", bufs=3, space="PSUM") as psum2,
        ):
            # all-ones [128, 1] bf16 for the tap-reduction matmul
            ones = opool.tile([128, 1], bf16, name="ones")
            nc.gpsimd.memset(ones[:], 1.0)

            # ---- input DMAs up-front ----
            # small z/w loads on the SWDGE ring so szw never gates matmuls
            wt = zpool.tile([128, 2], f32, name="wt")
            nc.gpsimd.dma_start(wt[:], w_in.rearrange("(k c) -> c k", c=128))
            zts = []
            for s in range(SPC):
                zt = zpool.tile([128, 2, KS * KS], f32, tag="zt", name=f"zt{s}")
                nc.gpsimd.dma_start(
                    zt[:], z_in[s].rearrange("(k c) p q -> c k (p q)", c=128)
                )
                zts.append(zt)
            w64 = zpool.tile([128, 2], f32, name="w64")
            nc.vector.tensor_scalar_mul(w64[:], wt[:], 1.0 / (KS * KS))
            # x pieces on the Sync HWDGE ring, in consumption order
            xst = {}
            for s in range(SPC):
                for pi, (b0, nbk) in enumerate(PIECES):
                    for k in range(2):
                        lo = b0 * BLK
                        ln = min(nbk * BLK, F - lo)
                        t = xstage.tile([128, 4 * BLK], f32, tag="xst",
                                        name=f"xst{s}{k}{pi}")
                        nc.sync.dma_start(t[:, :ln], xflat[s, k, :, lo : lo + ln])
                        xst[(s, k, pi)] = t

            # ---- stage 1 ----
            obufs = []
            for s in range(SPC):
                obuf = opool.tile([1, 3584], f32, tag=f"ob{s}", name=f"obuf{s}")
                obufs.append(obuf)
                zsq = zpool.tile([128, 2, KS * KS], f32, tag="zsq", name=f"zsq{s}")
                szw = zpool.tile([128, 2, KS * KS], bf16, tag="szw", name=f"szw{s}")

                for pi, (b0, nbk) in enumerate(PIECES):
                    lo = b0 * BLK
                    ln = min(nbk * BLK, F - lo)
                    # sqrt pieces (bf16); separate tiles per piece
                    sxp = {}
                    for k in range(2):
                        t = sxq.tile([128, 4 * BLK], bf16, tag="sxp",
                                     name=f"sxp{s}{k}{pi}")
                        nc.scalar.activation(
                            t[:, :ln], xst[(s, k, pi)][:, :ln], AF.Sqrt
                        )
                        sxp[k] = t
                    if pi == 0:
                        # z path: szw[c, k, t] = w[c]/64 * sqrt(z[c, t]);
                        # emitted after the first sqrts so it doesn't block
                        # ACT's queue head while waiting on the z/w loads
                        for k in range(2):
                            nc.scalar.activation(
                                zsq[:, k, :], zts[s][:, k, :], AF.Sqrt
                            )
                            nc.vector.tensor_scalar_mul(
                                szw[:, k, :], zsq[:, k, :], w64[:, k : k + 1]
                            )
                    # matmuls for this piece's blocks
                    pl = plane.tile([64, 4 * BLK], bf16, tag=f"pl{pi}",
                                    name=f"pl{s}{pi}")
                    pss = [
                        psum.tile([64, BLK], f32, tag="ps", name=f"ps_{s}_{b0+j}")
                        for j in range(nbk)
                    ]
                    for k in range(2):
                        for j in range(nbk):
                            nb = min(BLK, ln - j * BLK)
                            nc.tensor.matmul(
                                pss[j][:, :nb],
                                szw[:, k, :],
                                sxp[k][:, j * BLK : j * BLK + nb],
                                start=(k == 0),
                                stop=(k == 1),
                            )
                    for j in range(nbk):
                        nb = min(BLK, ln - j * BLK)
                        nc.vector.tensor_copy(
                            pl[:, j * BLK : j * BLK + nb], pss[j][:, :nb]
                        )
                    # dumps to DRAM scratch (HWDGE); chunk tensor ci holds
                    # plane cols [1024*ci, ...), with overlapping coverage:
                    if pi == 0:
                        # piece 0 = cols [0, 2048)
                        nc.sync.dma_start(
                            scs[s][0][:, 0 : 3 * BLK], pl[:, 0 : 3 * BLK]
                        )
                        nc.sync.dma_start(
                            scs[s][1][:, 0 : 2 * BLK], pl[:, 2 * BLK : 4 * BLK]
                        )
                    elif pi == 1:
                        # piece 1 = cols [2048, 3584)
                        nc.sync.dma_start(scs[s][1][:, 2 * BLK : 3 * BLK],
                                          pl[:, 0:BLK])
                        nc.sync.dma_start(scs[s][2][:, 0 : 3 * BLK],
                                          pl[:, 0 : 3 * BLK])
                        nc.sync.dma_start(scs[s][3][:, 0:BLK],
                                          pl[:, 2 * BLK : 3 * BLK])
                    else:
                        # piece 2 = cols [3584, 3969)
                        nc.sync.dma_start(scs[s][3][:, BLK : BLK + ln],
                                          pl[:, :ln])

            # ---- stage 2, chunked ----
            for s in range(SPC):
                obuf = obufs[s]
                for ci, (u0, ulen) in enumerate(CHUNKS):
                    # gather with per-tap shift: A[t, u] = plane[t, u0+u+off(t)]
                    a2 = gath.tile([64, 2 * BLK], bf16, tag="a2",
                                   name=f"a2_{s}{ci}")
                    pit = PITS[ci]
                    src = bass.AP(
                        scs[s][ci][:].tensor,
                        0,
                        [[8 * pit + MS, 8], [pit + 1, 8], [1, ulen]],
                    )
                    if ci < 2:
                        nc.gpsimd.dma_start(a2[:, :ulen], src)
                    else:
                        nc.sync.dma_start(a2[:, :ulen], src)

                    # tap reduction: o[u] = sum_t A[t, u]
                    ps2 = psum2.tile([1, 2 * BLK], f32, tag="ps2",
                                     name=f"ps2_{s}{ci}")
                    for b in range((ulen + BLK - 1) // BLK):
                        nb = min(BLK, ulen - b * BLK)
                        nc.tensor.matmul(
                            ps2[:, b * BLK : b * BLK + nb],
                            ones[0:64, :],
                            a2[:, b * BLK : b * BLK + nb],
                            start=True,
                            stop=True,
                        )
                    hl = (ulen // 2) & ~1
                    nc.scalar.copy(obuf[0:1, u0 : u0 + hl], ps2[:, :hl])
                    nc.vector.tensor_copy(
                        obuf[0:1, u0 + hl : u0 + ulen], ps2[:, hl:ulen]
                    )

                # extract valid rows: out[i, j] = o[63 i + j]; emitted as two
                # row-range DMAs so the first 32 rows (chunks 0-1 only) ship
                # before the last chunks finish
                osrc = obuf[0:1, 0 : MO * MS].rearrange("p (i j) -> p i j", i=MO)[
                    :, :, 0:MO
                ]
                nc.scalar.dma_start(out[s, 0, 0:32].unsqueeze(0), osrc[:, 0:32])
                nc.scalar.dma_start(out[s, 0, 32:MO].unsqueeze(0), osrc[:, 32:MO])

    nc.compile()
    return nc


def _get_nc():
    if "nc" not in _CACHE:
        _CACHE["nc"] = _build()
    return _CACHE["nc"]


def _run(z, x, weights, **runkw):
    z = np.ascontiguousarray(np.asarray(z), dtype=np.float32)
    x = np.ascontiguousarray(np.asarray(x), dtype=np.float32)
    w = np.ascontiguousarray(np.asarray(weights), dtype=np.float32).reshape(C)
    in_maps = []
    for i in range(NCORES):
        lo, hi = i * SPC, (i + 1) * SPC
        in_maps.append({"z": z[lo:hi], "x": x[lo:hi], "w": w})
    nc = _get_nc()
    try:
        res = run_bass_kernel_spmd(
            nc, in_maps, core_ids=list(range(NCORES)), **runkw
        )
    except Exception:
        # transient device errors (e.g. NRT exec-unit unrecoverable) have
        # been observed to succeed on retry
        res = run_bass_kernel_spmd(
            nc, in_maps, core_ids=list(range(NCORES)), **runkw
        )
    full = np.concatenate([res.results[i]["out"] for i in range(NCORES)], axis=0)
    return full, res


def kernel(z, x, weights):
    full, _ = _run(z, x, weights)
    return full


# revision 32
# speedup vs baseline: 1.0145x; 1.0145x over previous
"""Bhattacharyya coefficient kernel for Trainium2 (8 NeuronCores, SPMD).

out[n,0,i,j] = (1/k^2) * sum_{c,p,q} w[c] * sqrt(x[n,c,i+p,j+q] * z[n,c,p,q])

Data-parallel over batch: 2 samples per core. Per sample:
  1. ACT: sx = sqrt(x) (bf16), szw = w/k^2 * sqrt(z) (bf16).
  2. TensorE: plane[t, y] = sum_c szw[c, t] * sx[c, y] for the 64 taps
     t = 8p+q and all 63*63 image pixels y (K=256 in two 128-chunks
     accumulated in PSUM, M=64 taps, N in blocks of <=512).
  3. Evict PSUM -> SBUF (bf16, DVE), dump plane pieces to DRAM scratch.
  4. Gather back with per-tap shifted offsets (flat DRAM AP):
     A[t, u] = plane[t, u + 63*(t>>3) + (t&7)], turning the tap-sum into a
     pure partition reduction; chunked so each gather only needs the dumps
     that cover its column range.
  5. TensorE ones-matmul: o[u] = sum_t A[t, u]; evict; out[i,j] = o[63i+j].

The x loads are split into pieces (1MB, 0.79MB, 0.2MB per c-chunk) issued
up-front on the Sync HWDGE ring; the final tiny piece keeps the end-of-
kernel serial chain (last DMA -> sqrt -> matmul -> evict -> dump -> gather
-> reduce -> out) short.
"""

import numpy as np

import concourse.bacc as bacc
import concourse.bass as bass
import concourse.mybir as mybir
from concourse import tile
from concourse.bass_utils import run_bass_kernel_spmd

N, C, KS, MS = 16, 256, 8, 63
MO = MS - KS + 1            # 56
F = MS * MS                 # 3969
L = (MO - 1) * MS + MO + 2  # 3522 (even; last needed flat index is 63*55+55)
NCORES = 8
SPC = N // NCORES           # samples per core
BLK = 512
AF = mybir.ActivationFunctionType
f32 = mybir.dt.float32
bf16 = mybir.dt.bfloat16

# x staging pieces (start block, n blocks): 4 + 3 + 1
PIECES = [(0, 4), (4, 3), (7, 1)]
# stage-2 gather chunks (u0, len): chunk i>=1 reads scratch region 1
CHUNKS = [(0, 2 * BLK), (2 * BLK, 2 * BLK), (4 * BLK, 2 * BLK),
          (6 * BLK, L - 6 * BLK)]

_CACHE = {}


def _build():
    nc = bacc.Bacc("TRN2", target_bir_lowering=False, debug=False)
    z_in = nc.declare_dram_parameter("z", [SPC, C, KS, KS], f32, isOutput=False)
    x_in = nc.declare_dram_parameter("x", [SPC, C, MS, MS], f32, isOutput=False)
    w_in = nc.declare_dram_parameter("w", [C], f32, isOutput=False)
    out = nc.declare_dram_parameter("out", [SPC, 1, MO, MO], f32, isOutput=True)

    # DRAM scratch: one tensor per gather chunk (per sample) so Tile's
    # tensor-granular DRAM dependency tracking makes each gather wait only
    # for the dumps covering its own column range. Chunk tensor ci holds
    # plane cols [1024*ci, 1024*ci + pitch).
    PITS = [3 * BLK, 3 * BLK, 3 * BLK, 904]
    scs = [
        [nc.dram_tensor(f"pl_sc{ci}_{s}", [64, PITS[ci]], bf16)
         for ci in range(4)]
        for s in range(SPC)
    ]

    xflat = x_in.rearrange("s (k c) h w -> s k c (h w)", c=128)  # [SPC,2,128,F]

    with tile.TileContext(nc) as tc:
        with (
            tc.tile_pool(name="xstage", bufs=10) as xstage,
            tc.tile_pool(name="sxq", bufs=9) as sxq,
            tc.tile_pool(name="zpool", bufs=2) as zpool,
            tc.tile_pool(name="plane", bufs=2) as plane,
            tc.tile_pool(name="gath", bufs=6) as gath,
            tc.tile_pool(name="opool", bufs=1) as opool,
            tc.tile_pool(name="psum", bufs=2, space="PSUM") as psum,
            tc.tile_pool(name="psum2", bufs=3, space="PSUM") as psum2,
        ):
            # all-ones [128, 1] bf16 for the tap-reduction matmul
            ones = opool.tile([128, 1], bf16, name="ones")
            nc.gpsimd.memset(ones[:], 1.0)

            # ---- input DMAs up-front ----
            # small z/w loads on the SWDGE ring so szw never gates matmuls
            wt = zpool.tile([128, 2], f32, name="wt")
            nc.gpsimd.dma_start(wt[:], w_in.rearrange("(k c) -> c k", c=128))
            zts = []
            for s in range(SPC):
                zt = zpool.tile([128, 2, KS * KS], f32, tag="zt", name=f"zt{s}")
                nc.gpsimd.dma_start(
                    zt[:], z_in[s].rearrange("(k c) p q -> c k (p q)", c=128)
                )
                zts.append(zt)
            w64 = zpool.tile([128, 2], f32, name="w64")
            nc.vector.tensor_scalar_mul(w64[:], wt[:], 1.0 / (KS * KS))
            # x pieces on the Sync HWDGE ring, in consumption order
            xst = {}
            for s in range(SPC):
                for pi, (b0, nbk) in enumerate(PIECES):
                    for k in range(2):
                        lo = b0 * BLK
                        ln = min(nbk * BLK, F - lo)
                        t = xstage.tile([128, 4 * BLK], f32, tag="xst",
                                        name=f"xst{s}{k}{pi}")
                        nc.sync.dma_start(t[:, :ln], xflat[s, k, :, lo : lo + ln])
                        xst[(s, k, pi)] = t

            # ---- stage 1 ----
            obufs = []
            for s in range(SPC):
                obuf = opool.tile([1, 3584], f32, tag=f"ob{s}", name=f"obuf{s}")
                obufs.append(obuf)
                zsq = zpool.tile([128, 2, KS * KS], f32, tag="zsq", name=f"zsq{s}")
                szw = zpool.tile([128, 2, KS * KS], bf16, tag="szw", name=f"szw{s}")

                for pi, (b0, nbk) in enumerate(PIECES):
                    lo = b0 * BLK
                    ln = min(nbk * BLK, F - lo)
                    # sqrt pieces (bf16); separate tiles per piece
                    sxp = {}
                    for k in range(2):
                        t = sxq.tile([128, 4 * BLK], bf16, tag="sxp",
                                     name=f"sxp{s}{k}{pi}")
                        nc.scalar.activation(
                            t[:, :ln], xst[(s, k, pi)][:, :ln], AF.Sqrt
                        )
                        sxp[k] = t
                    if pi == 0:
                        # z path: szw[c, k, t] = w[c]/64 * sqrt(z[c, t]);
                        # emitted after the first sqrts so it doesn't block
                        # ACT's queue head while waiting on the z/w loads
                        for k in range(2):
                            nc.scalar.activation(
                                zsq[:, k, :], zts[s][:, k, :], AF.Sqrt
                            )
                            nc.vector.tensor_scalar_mul(
                                szw[:, k, :], zsq[:, k, :], w64[:, k : k + 1]
                            )
                    # matmuls for this piece's blocks
                    pl = plane.tile([64, 4 * BLK], bf16, tag=f"pl{pi}",
                                    name=f"pl{s}{pi}")
                    pss = [
                        psum.tile([64, BLK], f32, tag="ps", name=f"ps_{s}_{b0+j}")
                        for j in range(nbk)
                    ]
                    for k in range(2):
                        for j in range(nbk):
                            nb = min(BLK, ln - j * BLK)
                            nc.tensor.matmul(
                                pss[j][:, :nb],
                                szw[:, k, :],
                                sxp[k][:, j * BLK : j * BLK + nb],
                                start=(k == 0),
                                stop=(k == 1),
                            )
                    for j in range(nbk):
                        nb = min(BLK, ln - j * BLK)
                        nc.vector.tensor_copy(
                            pl[:, j * BLK : j * BLK + nb], pss[j][:, :nb]
                        )
                    # dumps to DRAM scratch (HWDGE); chunk tensor ci holds
                    # plane cols [1024*ci, ...), with overlapping coverage:
                    if pi == 0:
                        # piece 0 = cols [0, 2048)
                        nc.sync.dma_start(
                            scs[s][0][:, 0 : 3 * BLK], pl[:, 0 : 3 * BLK]
                        )
                        nc.sync.dma_start(
                            scs[s][1][:, 0 : 2 * BLK], pl[:, 2 * BLK : 4 * BLK]
                        )
                    elif pi == 1:
                        # piece 1 = cols [2048, 3584)
                        nc.sync.dma_start(scs[s][1][:, 2 * BLK : 3 * BLK],
                                          pl[:, 0:BLK])
                        nc.sync.dma_start(scs[s][2][:, 0 : 3 * BLK],
                                          pl[:, 0 : 3 * BLK])
                        nc.sync.dma_start(scs[s][3][:, 0:BLK],
                                          pl[:, 2 * BLK : 3 * BLK])
                    else:
                        # piece 2 = cols [3584, 3969)
                        nc.sync.dma_start(scs[s][3][:, BLK : BLK + ln],
                                          pl[:, :ln])

            # ---- stage 2, chunked ----
            for s in range(SPC):
                obuf = obufs[s]
                for ci, (u0, ulen) in enumerate(CHUNKS):
                    # gather with per-tap shift: A[t, u] = plane[t, u0+u+off(t)]
                    a2 = gath.tile([64, 2 * BLK], bf16, tag="a2",
                                   name=f"a2_{s}{ci}")
                    pit = PITS[ci]
                    src = bass.AP(
                        scs[s][ci][:].tensor,
                        0,
                        [[8 * pit + MS, 8], [pit + 1, 8], [1, ulen]],
                    )
                    if ci < 2:
                        nc.gpsimd.dma_start(a2[:, :ulen], src)
                    else:
                        nc.sync.dma_start(a2[:, :ulen], src)

                    # tap reduction: o[u] = sum_t A[t, u]
                    ps2 = psum2.tile([1, 2 * BLK], f32, tag="ps2",
                                     name=f"ps2_{s}{ci}")
                    for b in range((ulen + BLK - 1) // BLK):
                        nb = min(BLK, ulen - b * BLK)
                        nc.tensor.matmul(
                            ps2[:, b * BLK : b * BLK + nb],
                            ones[0:64, :],
                            a2[:, b * BLK : b * BLK + nb],
                            start=True,
                            stop=True,
                        )
                    hl = (ulen // 2) & ~1
                    nc.scalar.copy(obuf[0:1, u0 : u0 + hl], ps2[:, :hl])
                    nc.vector.tensor_copy(
                        obuf[0:1, u0 + hl : u0 + ulen], ps2[:, hl:ulen]
                    )

                # extract valid rows: out[i, j] = o[63 i + j]; emitted as two
                # row-range DMAs so the first 32 rows (chunks 0-1 only) ship
                # before the last chunks finish
                osrc = obuf[0:1, 0 : MO * MS].rearrange("p (i j) -> p i j", i=MO)[
                    :, :, 0:MO
                ]
                nc.scalar.dma_start(out[s, 0, 0:32].unsqueeze(0), osrc[:, 0:32])
                nc.scalar.dma_start(out[s, 0, 32:MO].unsqueeze(0), osrc[:, 32:MO])

    nc.compile()
    return nc


def _get_nc():
    if "nc" not in _CACHE:
        _CACHE["nc"] = _build()
    return _CACHE["nc"]


def _run(z, x, weights, **runkw):
    z = np.ascontiguousarray(np.asarray(z), dtype=np.float32)
    x = np.ascontiguousarray(np.asarray(x), dtype=np.float32)
    w = np.ascontiguousarray(np.asarray(weights), dtype=np.float32).reshape(C)
    in_maps = []
    for i in range(NCORES):
        lo, hi = i * SPC, (i + 1) * SPC
        in_maps.append({"z": z[lo:hi], "x": x[lo:hi], "w": w})
    nc = _get_nc()
    try:
        res = run_bass_kernel_spmd(
            nc, in_maps, core_ids=list(range(NCORES)), **runkw
        )
    except Exception:
        # transient device errors (e.g. NRT exec-unit unrecoverable) have
        # been observed to succeed on retry
        res = run_bass_kernel_spmd(
            nc, in_maps, core_ids=list(range(NCORES)), **runkw
        )
    full = np.concatenate([res.results[i]["out"] for i in range(NCORES)], axis=0)
    return full, res


def kernel(z, x, weights):
    full, _ = _run(z, x, weights)
    return full
